# revision 1
# baseline (speedup 1.0000x reference)
"""Self-contained Trainium2 Bass kernel: causal self-attention, 8-core SPMD.

nn_CausalSelfAttention: B=4, T=2048, C=1024, n_head=16 (fp32 reference).

Sharding (hardcoded): core c -> batch b = c//2, head-group g = c%2
(8 of 16 heads = 512 features). Data parallel over B, tensor parallel
over heads. Each core computes a partial output [T, C] = y_g @ Wp_g^T;
the host sums the two partials per batch and adds bp (the tensor-parallel
all-reduce done at unshard time).

Device kernel (per core, fused over 4 tq-blocks of 512):
  stage A: QKV projections (fp16 matmuls, fp32 PSUM accumulation),
           interleaved with stage B per head-pair for engine overlap
  stage B: flash-style attention in S^T layout ([ts=128, tq=512] tiles,
           2 heads row-packed per [128,1024] PSUM group, one Exp per
           group on ScalarE, post-exp causal mask multiply on VectorE,
           AV matmuls with a [v | 1] stationary operand (M=65) so row 64
           accumulates the softmax denominator), normalization via
           DRAM-bounce partition broadcast of 1/denominator
  stage C: output projection
Host-side prep is layout/sharding only (transposes into SBUF-image
layouts, slicing, cast to fp16); all FLOPs run on device. No on-chip
transposes needed. ~5e-4 relative error vs the fp32 reference.
"""

import sys

for _p in ("/opt/trn_rl_repo",):
    if _p not in sys.path:
        sys.path.insert(0, _p)

import numpy as np

import concourse.bacc as bacc
import concourse.bass as bass
import concourse.tile as tile
from concourse import mybir

F32 = mybir.dt.float32
F32R = mybir.dt.float32r

T = 2048
C = 1024
O = 512          # per-core output features (8 heads x 64)
HD = 64
NJB = 4          # tq blocks of 512
NTS = 16         # ts tiles of 128
NCC = 8          # c chunks of 128
NOC = 4          # o chunks of 128
SCALE = 1.0 / 8.0  # 1/sqrt(64)


def build(mm_mode: str = "f16"):
    """Returns (nc, meta). mm_mode in {'f32r', 'f16', 'bf16'}."""
    if mm_mode == "f32r":
        # tiles feeding matmuls must be *typed* float32r end-to-end (the BIR
        # verifier requires producers to be "rounded to FP32r")
        sb_dt = F32R
        np_dt = np.float32
    elif mm_mode == "bf16":
        import ml_dtypes
        sb_dt = mybir.dt.bfloat16
        np_dt = ml_dtypes.bfloat16
    elif mm_mode == "f16":
        sb_dt = mybir.dt.float16
        np_dt = np.float16
    else:
        raise ValueError(mm_mode)

    nc = bacc.Bacc("TRN2", target_bir_lowering=False, debug=False)

    # all large inputs come as SBUF images ([128 partitions, ...]) so each
    # DMA reads 128 contiguous multi-KB rows instead of 1024 strided 1KB rows
    xt_d = nc.dram_tensor("xt", [NJB, 128, NCC, 512], sb_dt, kind="ExternalInput").ap()
    wqt_d = nc.dram_tensor("wqt", [128, NCC, O], sb_dt, kind="ExternalInput").ap()
    wkt_d = nc.dram_tensor("wkt", [128, NCC, O], sb_dt, kind="ExternalInput").ap()
    wvt_d = nc.dram_tensor("wvt", [128, NCC, O], sb_dt, kind="ExternalInput").ap()
    wpt_d = nc.dram_tensor("wpt", [128, NOC, C], sb_dt, kind="ExternalInput").ap()
    bq_d = nc.dram_tensor("bq", [128, NOC], F32, kind="ExternalInput").ap()
    bk_d = nc.dram_tensor("bk", [128, NOC], F32, kind="ExternalInput").ap()
    bvb_d = nc.dram_tensor("bvb", [128, O], F32, kind="ExternalInput").ap()
    mask_d = nc.dram_tensor("masks", [128, 4, 512], sb_dt, kind="ExternalInput").ap()
    out_d = nc.dram_tensor("out", [T, C], F32, kind="ExternalOutput").ap()
    # denominator bounce buffer for partition-broadcast
    dscr_d = nc.dram_tensor("dscr", [NJB, 4, 2, 512], F32, kind="Internal").ap()

    with tile.TileContext(nc) as tc:
        with (
            tc.tile_pool(name="const", bufs=1) as const,
            tc.tile_pool(name="xt_pool", bufs=2) as xt_pool,
            tc.tile_pool(name="qt_pool", bufs=2) as qt_pool,
            tc.tile_pool(name="att_pool", bufs=4) as att_pool,
            tc.tile_pool(name="yt_pool", bufs=2) as yt_pool,
            tc.tile_pool(name="misc", bufs=2) as misc,
            tc.tile_pool(name="bc_pool", bufs=2) as bc_pool,
            tc.tile_pool(name="ost_pool", bufs=3) as ost_pool,
            tc.tile_pool(name="pst", bufs=2, space="PSUM") as pst,
            tc.tile_pool(name="pa", bufs=2, space="PSUM") as pa,
            tc.tile_pool(name="pav", bufs=2, space="PSUM") as pav,
        ):
            # ---- constants / weights (resident) ----
            wq_sb = const.tile([128, NCC, O], sb_dt, name="wq_sb")
            wk_sb = const.tile([128, NCC, O], sb_dt, name="wk_sb")
            wv_sb = const.tile([128, NCC, O], sb_dt, name="wv_sb")
            wp_sb = const.tile([128, NOC, C], sb_dt, name="wp_sb")
            nc.sync.dma_start(out=wq_sb, in_=wqt_d)
            nc.scalar.dma_start(out=wk_sb, in_=wkt_d)
            nc.gpsimd.dma_start(out=wv_sb, in_=wvt_d)
            nc.scalar.dma_start(out=wp_sb, in_=wpt_d)

            bq_sb = const.tile([128, NOC], F32, name="bq_sb")
            bk_sb = const.tile([128, NOC], F32, name="bk_sb")
            nc.gpsimd.dma_start(out=bq_sb, in_=bq_d)
            nc.gpsimd.dma_start(out=bk_sb, in_=bk_d)
            bvb_sb = const.tile([128, O], F32, name="bvb_sb")
            nc.gpsimd.dma_start(out=bvb_sb, in_=bvb_d)

            mask_sb = const.tile([128, 4, 512], sb_dt, name="mask_sb")
            nc.gpsimd.dma_start(out=mask_sb, in_=mask_d)

            # persistent K^T and V, as per-(chunk, block) tiles so stage A of
            # block jb+1 has no false WAR deps against stage B reads of jb.
            # V carries a ones column per head ([v | 1]) so the AV matmul
            # (M=65) also accumulates the softmax denominator in its row 64.
            kt_t = {}
            v_t = {}
            for jbx in range(NJB):
                for oc in range(NOC):
                    kt_t[oc, jbx] = const.tile(
                        [128, 512], sb_dt, name=f"kt{oc}_{jbx}"
                    )
                v_t[jbx] = const.tile([128, 4, 8, 65], sb_dt, name=f"v_{jbx}")
                ones_col = v_t[jbx][:, :, :, 64:65]
                if sb_dt == F32R:
                    ones_col = ones_col.bitcast(F32)
                nc.vector.memset(ones_col, 1.0)

            for jb in range(NJB):
                # ---- stage A: QKV projections for t-block jb ----
                xt_a = xt_pool.tile([128, 4, 512], sb_dt, tag="xta", name="xt_a")
                xt_b = xt_pool.tile([128, 4, 512], sb_dt, tag="xtb", name="xt_b")
                xt_sb = (xt_a, xt_b)
                nc.sync.dma_start(out=xt_a, in_=xt_d[jb, :, 0:4])
                nc.scalar.dma_start(out=xt_b, in_=xt_d[jb, :, 4:8])

                qt_sb = qt_pool.tile([128, NOC, 512], sb_dt, tag="qt")

                def emit_qk(oc):
                    # q/k projections for o-chunk oc; evictions on ScalarE
                    for mat, w_sb in ((0, wq_sb), (1, wk_sb)):
                        ps = pa.tile([128, 512], F32, tag="apsum", name=f"qk{oc}{mat}")
                        for cc in range(NCC):
                            nc.tensor.matmul(
                                ps,
                                lhsT=w_sb[:, cc, 128 * oc : 128 * (oc + 1)],
                                rhs=xt_sb[cc // 4][:, cc % 4, :],
                                start=(cc == 0),
                                stop=(cc == NCC - 1),
                            )
                        if mat == 0:
                            nc.scalar.activation(
                                qt_sb[:, oc, :], ps,
                                mybir.ActivationFunctionType.Identity,
                                bias=bq_sb[:, oc : oc + 1], scale=SCALE,
                            )
                        else:
                            nc.scalar.activation(
                                kt_t[oc, jb], ps,
                                mybir.ActivationFunctionType.Identity,
                                bias=bk_sb[:, oc : oc + 1], scale=1.0,
                            )

                def emit_v():
                    # v: out layout [t-part, o]; lhsT = xt chunk, rhs = wv
                    for tt in range(4):
                        ps = pa.tile([128, 512], F32, tag="apsum", name=f"v{tt}")
                        for cc in range(NCC):
                            nc.tensor.matmul(
                                ps,
                                lhsT=xt_sb[cc // 4][
                                    :, cc % 4, 128 * tt : 128 * (tt + 1)
                                ],
                                rhs=wv_sb[:, cc, :],
                                start=(cc == 0),
                                stop=(cc == NCC - 1),
                            )
                        nc.vector.scalar_tensor_tensor(
                            v_t[jb][:, tt, :, 0:64],
                            ps.rearrange("p (h d) -> p h d", h=8),
                            0.0,
                            bvb_sb.rearrange("p (h d) -> p h d", h=8),
                            op0=mybir.AluOpType.add,
                            op1=mybir.AluOpType.add,
                        )

                # ---- stages A+B interleaved: qk(p) [+v after qk(0)] then
                # attention pair p, so ScalarE gets exp work almost
                # immediately at each t-block boundary ----
                yt_c = [
                    yt_pool.tile([128, 512], sb_dt, tag=f"yt{_o}", name=f"yt{_o}")
                    for _o in range(NOC)
                ]
                for p in range(4):  # head pairs == o-chunks
                    emit_qk(p)
                    if p == 0:
                        emit_v()
                    avpa = pav.tile([128, 512], F32, tag="av", name=f"avpa{p}")
                    avpb = pav.tile([128, 512], F32, tag="av", name=f"avpb{p}")
                    n_ts = 4 * jb + 4
                    for tsb in range(n_ts):
                        first = tsb == 0
                        last = tsb == n_ts - 1
                        st = pst.tile([128, 1024], F32, tag="st")
                        for r2 in range(2):
                            nc.tensor.matmul(
                                st[:, 512 * r2 : 512 * (r2 + 1)],
                                lhsT=kt_t[p, tsb // 4][
                                    64 * r2 : 64 * (r2 + 1),
                                    128 * (tsb % 4) : 128 * (tsb % 4 + 1),
                                ],
                                rhs=qt_sb[64 * r2 : 64 * (r2 + 1), p, :],
                                tile_position=(64 * r2, 0),
                                start=True,
                                stop=True,
                            )
                        att = att_pool.tile([128, 1024], sb_dt, tag="att")
                        nc.scalar.activation(
                            att, st, mybir.ActivationFunctionType.Exp
                        )
                        if tsb >= 4 * jb:  # diagonal tile: causal mask
                            r = tsb - 4 * jb
                            for r2 in range(2):
                                sl5 = slice(512 * r2, 512 * (r2 + 1))
                                nc.vector.tensor_mul(
                                    att[:, sl5], att[:, sl5], mask_sb[:, r, :]
                                )
                        for r2, avp in ((0, avpa), (1, avpb)):
                            h = 2 * p + r2
                            nc.tensor.matmul(
                                avp[0:65, :],
                                lhsT=v_t[tsb // 4][:, tsb % 4, h, :],
                                rhs=att[:, 512 * r2 : 512 * (r2 + 1)],
                                start=first,
                                stop=last,
                            )
                    # evict unnormalized y^T + denominators to SBUF at once
                    # so the PSUM banks free up for the next pair immediately
                    yra = misc.tile([65, 512], F32, tag="yra", name=f"yra{p}")
                    yrb = misc.tile([65, 512], F32, tag="yrb", name=f"yrb{p}")
                    nc.vector.tensor_copy(yra, avpa[0:65, :])
                    nc.vector.tensor_copy(yrb, avpb[0:65, :])
                    # normalization: denom -> DRAM bounce -> partition bcast
                    den2 = misc.tile([33, 1024], F32, tag="recip")
                    nc.vector.memset(den2[:, 0:512], 1.0)
                    nc.vector.tensor_copy(den2[0:1, 0:512], yra[64:65, :])
                    nc.vector.tensor_copy(den2[32:33, 0:512], yrb[64:65, :])
                    nc.vector.reciprocal(den2[0:33, 512:1024], den2[0:33, 0:512])
                    nc.gpsimd.dma_start(out=dscr_d[jb, p, 0], in_=den2[0:1, 512:1024])
                    nc.gpsimd.dma_start(out=dscr_d[jb, p, 1], in_=den2[32:33, 512:1024])
                    bca = bc_pool.tile([64, 512], F32, tag="bca", name=f"bca{p}")
                    bcb = bc_pool.tile([64, 512], F32, tag="bcb", name=f"bcb{p}")
                    for bt, hh in ((bca, 0), (bcb, 1)):
                        srcp = dscr_d[jb, p, hh]
                        bcast_ap = bass.AP(
                            tensor=srcp.tensor,
                            offset=srcp.offset,
                            ap=[[0, 64], [1, 512]],
                        )
                        nc.gpsimd.dma_start(out=bt, in_=bcast_ap)
                    nc.vector.tensor_mul(yt_c[p][0:64, :], yra[0:64, :], bca)
                    nc.vector.tensor_mul(yt_c[p][64:128, :], yrb[0:64, :], bcb)

                # ---- stage C: output projection for t-block jb ----
                for cb in range(2):
                    for tt in range(4):
                        op = pav.tile([128, 512], F32, tag="av", name="op_ps")
                        for oc in range(NOC):
                            nc.tensor.matmul(
                                op,
                                lhsT=yt_c[oc][:, 128 * tt : 128 * (tt + 1)],
                                rhs=wp_sb[:, oc, 512 * cb : 512 * (cb + 1)],
                                start=(oc == 0),
                                stop=(oc == NOC - 1),
                            )
                        ost = ost_pool.tile([128, 512], F32, tag="ost")
                        nc.vector.tensor_copy(ost, op)
                        nc.gpsimd.dma_start(
                            out=out_d[
                                512 * jb + 128 * tt : 512 * jb + 128 * (tt + 1),
                                512 * cb : 512 * (cb + 1),
                            ],
                            in_=ost,
                        )

    nc.finalize()
    return nc, {"np_dt": np_dt}


def make_masks(np_dt):
    """masks[r][p, n] = 1 if n >= 128*r + p else 0."""
    n = np.arange(512)[None, :]
    p = np.arange(128)[:, None]
    out = np.zeros((4, 128, 512), np.float32)
    for r in range(4):
        out[r] = (n >= 128 * r + p).astype(np.float32)
    return out.astype(np_dt)


def shard_inputs(inputs, np_dt):
    """Full inputs -> list of 8 per-core input dicts."""
    q = np.asarray(inputs["query"], np.float32)
    Wq = np.asarray(inputs["Wq"], np.float32)
    Wk = np.asarray(inputs["Wk"], np.float32)
    Wv = np.asarray(inputs["Wv"], np.float32)
    Wp = np.asarray(inputs["Wp"], np.float32)
    bq = np.asarray(inputs["bq"], np.float32)
    bk = np.asarray(inputs["bk"], np.float32)
    bv = np.asarray(inputs["bv"], np.float32)
    masks = np.ascontiguousarray(make_masks(np_dt).transpose(1, 0, 2))
    in_maps = []
    for core in range(8):
        b, g = core // 2, core % 2
        sl = slice(O * g, O * (g + 1))
        in_maps.append({
            "xt": np.ascontiguousarray(
                q[b].T.reshape(8, 128, NJB, 512).transpose(2, 1, 0, 3)
            ).astype(np_dt),
            "wqt": np.ascontiguousarray(
                Wq[sl, :].T.reshape(8, 128, O).transpose(1, 0, 2)
            ).astype(np_dt),
            "wkt": np.ascontiguousarray(
                Wk[sl, :].T.reshape(8, 128, O).transpose(1, 0, 2)
            ).astype(np_dt),
            "wvt": np.ascontiguousarray(
                Wv[sl, :].T.reshape(8, 128, O).transpose(1, 0, 2)
            ).astype(np_dt),
            "wpt": np.ascontiguousarray(
                Wp[:, sl].T.reshape(NOC, 128, C).transpose(1, 0, 2)
            ).astype(np_dt),
            "bq": np.ascontiguousarray(bq[sl].reshape(NOC, 128).T) * np.float32(SCALE),
            "bk": np.ascontiguousarray(bk[sl].reshape(NOC, 128).T),
            "bvb": np.broadcast_to(bv[sl], (128, O)).copy(),
            "masks": masks,
        })
    return in_maps


def unshard(results, bp):
    out = np.empty((4, T, C), np.float32)
    for b in range(4):
        out[b] = results[2 * b]["out"] + results[2 * b + 1]["out"] + np.asarray(
            bp, np.float32
        )
    return out


_CACHE = {}


def _get_nc(mode="f16"):
    if mode not in _CACHE:
        _CACHE[mode] = build(mode)
    return _CACHE[mode]


def kernel(**inputs):
    """Full unsharded inputs -> full [4, 2048, 1024] fp32 output."""
    from concourse import bass_utils

    nc, meta = _get_nc("f16")
    in_maps = shard_inputs(inputs, meta["np_dt"])
    res = bass_utils.run_bass_kernel_spmd(nc, in_maps, list(range(8)))
    return unshard(res.results, inputs["bp"])



# revision 2
# speedup vs baseline: 1.1525x; 1.1525x over previous
"""Self-contained Trainium2 Bass kernel: causal self-attention, 8-core SPMD.

nn_CausalSelfAttention: B=4, T=2048, C=1024, n_head=16 (fp32 reference).

Sharding (hardcoded): core c -> batch b = c//2, head-group g = c%2
(8 of 16 heads = 512 features). Data parallel over B, tensor parallel
over heads. Each core computes a partial output [T, C] = y_g @ Wp_g^T;
the host sums the two partials per batch and adds bp (the tensor-parallel
all-reduce done at unshard time).

Device kernel (per core), v2:
  stage A: QKV projections (fp16 matmuls, fp32 PSUM accumulation).
           Block jb+1's projections are emitted as PE filler interleaved
           into block jb's attention loop so the PE never starves while
           ScalarE computes exp (keeps the HAM clock-gate warm).
  stage B: flash-style attention in S^T layout ([ts=128, tq<=512] tiles,
           2 heads row-packed per [128,1024] PSUM group, one Exp per
           group on ScalarE, post-exp causal mask multiply on VectorE,
           AV matmuls with a [v | 1] stationary operand (M=65) so row 64
           accumulates the softmax denominator). Diagonal tiles are
           narrowed to tq >= 128*r (causal): less PE + exp work.
           Normalization: ScalarE extracts the denominator row,
           VectorE reciprocal_approx_fast, GPSIMD partition_broadcast,
           VectorE multiply -- all on-chip, no DRAM bounce.
  stage C: output projection
Host-side prep is layout/sharding only (transposes into SBUF-image
layouts, slicing, cast to fp16); all FLOPs run on device.
"""

import sys

for _p in ("/opt/trn_rl_repo",):
    if _p not in sys.path:
        sys.path.insert(0, _p)

import numpy as np

import concourse.bacc as bacc
import concourse.bass as bass
import concourse.tile as tile
from concourse import mybir

F32 = mybir.dt.float32
F16 = mybir.dt.float16

T = 2048
C = 1024
O = 512          # per-core output features (8 heads x 64)
HD = 64
NJB = 4          # tq blocks of 512
NCC = 8          # c chunks of 128
NOC = 4          # o chunks of 128
SCALE = 1.0 / 8.0  # 1/sqrt(64)


def build(mm_mode: str = "f16"):
    sb_dt = F16
    np_dt = np.float16

    nc = bacc.Bacc("TRN2", target_bir_lowering=False, debug=False)

    xt_d = nc.dram_tensor("xt", [NJB, 128, NCC, 512], sb_dt, kind="ExternalInput").ap()
    wqt_d = nc.dram_tensor("wqt", [NOC, 128, NCC, 128], sb_dt, kind="ExternalInput").ap()
    wkt_d = nc.dram_tensor("wkt", [NOC, 128, NCC, 128], sb_dt, kind="ExternalInput").ap()
    wvt_d = nc.dram_tensor("wvt", [NCC, 128, O], sb_dt, kind="ExternalInput").ap()
    wpt_d = nc.dram_tensor("wpt", [NOC, 128, C], sb_dt, kind="ExternalInput").ap()
    bq_d = nc.dram_tensor("bq", [128, NOC], F32, kind="ExternalInput").ap()
    bk_d = nc.dram_tensor("bk", [128, NOC], F32, kind="ExternalInput").ap()
    bvb_d = nc.dram_tensor("bvb", [128, O], F32, kind="ExternalInput").ap()
    mask_d = nc.dram_tensor("masks", [128, 4, 512], sb_dt, kind="ExternalInput").ap()
    out_d = nc.dram_tensor("out", [T, C], F32, kind="ExternalOutput").ap()

    with tile.TileContext(nc) as tc:
        with (
            tc.tile_pool(name="const", bufs=1) as const,
            tc.tile_pool(name="xt_pool", bufs=2) as xt_pool,
            tc.tile_pool(name="qt_pool", bufs=2) as qt_pool,
            tc.tile_pool(name="att_pool", bufs=4) as att_pool,
            tc.tile_pool(name="yt_pool", bufs=2) as yt_pool,
            tc.tile_pool(name="misc", bufs=2) as misc,
            tc.tile_pool(name="bc_pool", bufs=2) as bc_pool,
            tc.tile_pool(name="ost_pool", bufs=3) as ost_pool,
            tc.tile_pool(name="pst", bufs=2, space="PSUM") as pst,
            tc.tile_pool(name="pa", bufs=2, space="PSUM") as pa,
            tc.tile_pool(name="pav", bufs=1, space="PSUM") as pav,
        ):
            # ---- small constants first (they gate early compute) ----
            bq_sb = const.tile([128, NOC], F32, name="bq_sb")
            bk_sb = const.tile([128, NOC], F32, name="bk_sb")
            bvb_sb = const.tile([128, O], F32, name="bvb_sb")
            mask_sb = const.tile([128, 4, 512], sb_dt, name="mask_sb")
            nc.gpsimd.dma_start(out=bq_sb, in_=bq_d)
            nc.gpsimd.dma_start(out=bk_sb, in_=bk_d)
            nc.gpsimd.dma_start(out=bvb_sb, in_=bvb_d)
            nc.gpsimd.dma_start(out=mask_sb, in_=mask_d)

            # ---- x tiles (per block, two halves) on the sync queue ----
            xt_t = {}

            def load_xt(jb):
                xa = xt_pool.tile([128, 4, 512], sb_dt, tag="xta", name=f"xt{jb}a")
                xb = xt_pool.tile([128, 4, 512], sb_dt, tag="xtb", name=f"xt{jb}b")
                nc.sync.dma_start(out=xa, in_=xt_d[jb, :, 0:4])
                nc.sync.dma_start(out=xb, in_=xt_d[jb, :, 4:8])
                xt_t[jb] = (xa, xb)

            load_xt(0)

            # ---- weights: fine-grained tiles, arrival-ordered ----
            wq_t = [const.tile([128, NCC, 128], sb_dt, name=f"wq{o}") for o in range(NOC)]
            wk_t = [const.tile([128, NCC, 128], sb_dt, name=f"wk{o}") for o in range(NOC)]
            wv_t = [const.tile([128, O], sb_dt, name=f"wv{c}") for c in range(NCC)]
            wp_t = [const.tile([128, C], sb_dt, name=f"wp{o}") for o in range(NOC)]
            nc.gpsimd.dma_start(out=wq_t[0], in_=wqt_d[0])
            nc.gpsimd.dma_start(out=wk_t[0], in_=wkt_d[0])
            for cc in range(NCC):
                nc.gpsimd.dma_start(out=wv_t[cc], in_=wvt_d[cc])
            for oc in range(1, NOC):
                nc.gpsimd.dma_start(out=wq_t[oc], in_=wqt_d[oc])
                nc.gpsimd.dma_start(out=wk_t[oc], in_=wkt_d[oc])
            for oc in range(NOC):
                nc.gpsimd.dma_start(out=wp_t[oc], in_=wpt_d[oc])

            # persistent K^T and V. V carries a ones column per head
            # ([v | 1]) so the AV matmul (M=65) also accumulates the
            # softmax denominator in its row 64.
            kt_t = {}
            v_t = {}
            for jbx in range(NJB):
                for oc in range(NOC):
                    kt_t[oc, jbx] = const.tile(
                        [128, 512], sb_dt, name=f"kt{oc}_{jbx}"
                    )
                v_t[jbx] = const.tile([128, 4, 8, 65], sb_dt, name=f"v_{jbx}")
                nc.vector.memset(v_t[jbx][:, :, :, 64:65], 1.0)

            qt_sb = {}

            def emit_qk(jb, oc):
                # q and k projections for o-chunk oc of block jb;
                # evictions on VectorE (ScalarE is reserved for exp)
                if oc == 0:
                    qt_sb[jb] = qt_pool.tile(
                        [128, NOC, 512], sb_dt, tag="qt", name=f"qt{jb}"
                    )
                xab = xt_t[jb]
                for mat in (0, 1):
                    w = (wq_t if mat == 0 else wk_t)[oc]
                    ps = pa.tile([128, 512], F32, tag="apsum", name=f"qk{jb}{oc}{mat}")
                    for cc in range(NCC):
                        nc.tensor.matmul(
                            ps,
                            lhsT=w[:, cc, :],
                            rhs=xab[cc // 4][:, cc % 4, :],
                            start=(cc == 0),
                            stop=(cc == NCC - 1),
                        )
                    if mat == 0:
                        nc.vector.tensor_scalar(
                            out=qt_sb[jb][:, oc, :], in0=ps,
                            scalar1=SCALE, scalar2=bq_sb[:, oc : oc + 1],
                            op0=mybir.AluOpType.mult, op1=mybir.AluOpType.add,
                        )
                    else:
                        nc.vector.tensor_scalar(
                            out=kt_t[oc, jb], in0=ps,
                            scalar1=1.0, scalar2=bk_sb[:, oc : oc + 1],
                            op0=mybir.AluOpType.mult, op1=mybir.AluOpType.add,
                        )

            def emit_v(jb, tt):
                # v for t-chunk tt: out layout [t-part, o]
                xab = xt_t[jb]
                ps = pa.tile([128, 512], F32, tag="apsum", name=f"v{jb}{tt}")
                for cc in range(NCC):
                    nc.tensor.matmul(
                        ps,
                        lhsT=xab[cc // 4][:, cc % 4, 128 * tt : 128 * (tt + 1)],
                        rhs=wv_t[cc],
                        start=(cc == 0),
                        stop=(cc == NCC - 1),
                    )
                nc.vector.scalar_tensor_tensor(
                    v_t[jb][:, tt, :, 0:64],
                    ps.rearrange("p (h d) -> p h d", h=8),
                    0.0,
                    bvb_sb.rearrange("p (h d) -> p h d", h=8),
                    op0=mybir.AluOpType.add,
                    op1=mybir.AluOpType.add,
                )

            # PE filler: stage-A chains for the next block, drip-fed into
            # the attention loop so the PE queue never drains while ACT
            # works through the exps.
            filler = []

            def pop_filler():
                if filler:
                    filler.pop(0)()

            def attn_pair(jb, p, yt_c, fill_every):
                avp = pav.tile([65, 1024], F32, tag="av", name=f"av{jb}{p}")
                n_ts = 4 * jb + 4
                for tsb in range(n_ts):
                    first = tsb == 0
                    last = tsb == n_ts - 1
                    diag = tsb >= 4 * jb
                    r = tsb - 4 * jb
                    lo = 128 * r if diag else 0  # causal narrowing
                    st = pst.tile([128, 1024], F32, tag="st")
                    for r2 in range(2):
                        nc.tensor.matmul(
                            st[:, 512 * r2 + lo : 512 * (r2 + 1)],
                            lhsT=kt_t[p, tsb // 4][
                                64 * r2 : 64 * (r2 + 1),
                                128 * (tsb % 4) : 128 * (tsb % 4 + 1),
                            ],
                            rhs=qt_sb[jb][64 * r2 : 64 * (r2 + 1), p, lo:512],
                            tile_position=(64 * r2, 0),
                            start=True,
                            stop=True,
                        )
                    att = att_pool.tile([128, 1024], sb_dt, tag="att")
                    st3 = st.rearrange("p (h q) -> p h q", h=2)
                    att3 = att.rearrange("p (h q) -> p h q", h=2)
                    nc.scalar.activation(
                        att3[:, :, lo:512], st3[:, :, lo:512],
                        mybir.ActivationFunctionType.Exp,
                    )
                    if diag:
                        for r2 in range(2):
                            sl5 = slice(512 * r2 + lo, 512 * (r2 + 1))
                            nc.vector.tensor_mul(
                                att[:, sl5], att[:, sl5], mask_sb[:, r, lo:512]
                            )
                    for r2 in range(2):
                        h = 2 * p + r2
                        nc.tensor.matmul(
                            avp[0:65, 512 * r2 + lo : 512 * (r2 + 1)],
                            lhsT=v_t[tsb // 4][:, tsb % 4, h, :],
                            rhs=att[:, 512 * r2 + lo : 512 * (r2 + 1)],
                            start=first,
                            stop=last,
                        )
                    if tsb % fill_every == fill_every - 1:
                        pop_filler()
                # ---- normalization (all on-chip) ----
                den = misc.tile([1, 1024], F32, tag="den")
                rden = misc.tile([1, 1024], F32, tag="rden")
                nc.scalar.copy(den, avp[64:65, :])
                nc.vector.reciprocal_approx_fast(rden, den)
                bc = bc_pool.tile([64, 1024], F32, tag="bc")
                nc.gpsimd.partition_broadcast(bc, rden, channels=64)
                nc.vector.tensor_mul(yt_c[p][0:64, :], avp[0:64, 0:512], bc[:, 0:512])
                nc.vector.tensor_mul(
                    yt_c[p][64:128, :], avp[0:64, 512:1024], bc[:, 512:1024]
                )

            def stage_c(jb, yt_c):
                for cb in range(2):
                    for tt in range(4):
                        op = pa.tile([128, 512], F32, tag="apsum", name=f"c{cb}{tt}")
                        for oc in range(NOC):
                            nc.tensor.matmul(
                                op,
                                lhsT=yt_c[oc][:, 128 * tt : 128 * (tt + 1)],
                                rhs=wp_t[oc][:, 512 * cb : 512 * (cb + 1)],
                                start=(oc == 0),
                                stop=(oc == NOC - 1),
                            )
                        ost = ost_pool.tile([128, 512], F32, tag="ost")
                        nc.vector.tensor_copy(ost, op)
                        nc.sync.dma_start(
                            out=out_d[
                                512 * jb + 128 * tt : 512 * jb + 128 * (tt + 1),
                                512 * cb : 512 * (cb + 1),
                            ],
                            in_=ost,
                        )

            # ---- prologue: stage A for block 0 (v early: AV needs it) ----
            emit_qk(0, 0)
            for tt in range(4):
                emit_v(0, tt)
            for oc in range(1, NOC):
                emit_qk(0, oc)

            for jb in range(NJB):
                if jb + 1 < NJB:
                    load_xt(jb + 1)
                    for oc in range(NOC):
                        filler.append(lambda jbn=jb + 1, o=oc: emit_qk(jbn, o))
                    for tt in range(4):
                        filler.append(lambda jbn=jb + 1, t=tt: emit_v(jbn, t))
                yt_c = [
                    yt_pool.tile([128, 512], sb_dt, tag=f"yt{_o}", name=f"yt{_o}")
                    for _o in range(NOC)
                ]
                n_iters = 4 * (4 * jb + 4)
                fill_every = max(1, n_iters // 8) if jb + 1 < NJB else 10**9
                for p in range(NOC):
                    attn_pair(jb, p, yt_c, fill_every)
                while filler:
                    pop_filler()
                stage_c(jb, yt_c)

    nc.finalize()
    return nc, {"np_dt": np_dt}


def make_masks(np_dt):
    """masks[r][p, n] = 1 if n >= 128*r + p else 0."""
    n = np.arange(512)[None, :]
    p = np.arange(128)[:, None]
    out = np.zeros((4, 128, 512), np.float32)
    for r in range(4):
        out[r] = (n >= 128 * r + p).astype(np.float32)
    return out.astype(np_dt)


def shard_inputs(inputs, np_dt):
    """Full inputs -> list of 8 per-core input dicts."""
    q = np.asarray(inputs["query"], np.float32)
    Wq = np.asarray(inputs["Wq"], np.float32)
    Wk = np.asarray(inputs["Wk"], np.float32)
    Wv = np.asarray(inputs["Wv"], np.float32)
    Wp = np.asarray(inputs["Wp"], np.float32)
    bq = np.asarray(inputs["bq"], np.float32)
    bk = np.asarray(inputs["bk"], np.float32)
    bv = np.asarray(inputs["bv"], np.float32)
    masks = np.ascontiguousarray(make_masks(np_dt).transpose(1, 0, 2))
    in_maps = []
    for core in range(8):
        b, g = core // 2, core % 2
        sl = slice(O * g, O * (g + 1))
        in_maps.append({
            "xt": np.ascontiguousarray(
                q[b].T.reshape(8, 128, NJB, 512).transpose(2, 1, 0, 3)
            ).astype(np_dt),
            # [oc, p, cc, j]: lhsT chunk for (oc, cc) = W.T[128cc:+128, 128oc:+128]
            "wqt": np.ascontiguousarray(
                Wq[sl, :].T.reshape(NCC, 128, NOC, 128).transpose(2, 1, 0, 3)
            ).astype(np_dt),
            "wkt": np.ascontiguousarray(
                Wk[sl, :].T.reshape(NCC, 128, NOC, 128).transpose(2, 1, 0, 3)
            ).astype(np_dt),
            # [cc, p, o]
            "wvt": np.ascontiguousarray(
                Wv[sl, :].T.reshape(NCC, 128, O)
            ).astype(np_dt),
            # [oc, p, c]
            "wpt": np.ascontiguousarray(
                Wp[:, sl].T.reshape(NOC, 128, C)
            ).astype(np_dt),
            "bq": np.ascontiguousarray(bq[sl].reshape(NOC, 128).T) * np.float32(SCALE),
            "bk": np.ascontiguousarray(bk[sl].reshape(NOC, 128).T),
            "bvb": np.broadcast_to(bv[sl], (128, O)).copy(),
            "masks": masks,
        })
    return in_maps


def unshard(results, bp):
    out = np.empty((4, T, C), np.float32)
    for b in range(4):
        out[b] = results[2 * b]["out"] + results[2 * b + 1]["out"] + np.asarray(
            bp, np.float32
        )
    return out


_CACHE = {}


def _get_nc(mode="f16"):
    if mode not in _CACHE:
        _CACHE[mode] = build(mode)
    return _CACHE[mode]


def kernel(**inputs):
    """Full unsharded inputs -> full [4, 2048, 1024] fp32 output."""
    from concourse import bass_utils

    nc, meta = _get_nc("f16")
    in_maps = shard_inputs(inputs, meta["np_dt"])
    res = bass_utils.run_bass_kernel_spmd(nc, in_maps, list(range(8)))
    return unshard(res.results, inputs["bp"])


# revision 5
# speedup vs baseline: 1.2267x; 1.0643x over previous
"""Self-contained Trainium2 Bass kernel: causal self-attention, 8-core SPMD.

nn_CausalSelfAttention: B=4, T=2048, C=1024, n_head=16 (fp32 reference).

Sharding (hardcoded): core c -> batch b = c//2, head-group g = c%2
(8 of 16 heads = 512 features). Data parallel over B, tensor parallel
over heads. Each core computes a partial output [T, C] = y_g @ Wp_g^T;
the host sums the two partials per batch and adds bp (the tensor-parallel
all-reduce done at unshard time).

Device kernel (per core), v2:
  stage A: QKV projections (fp16 matmuls, fp32 PSUM accumulation).
           Block jb+1's projections are emitted as PE filler interleaved
           into block jb's attention loop so the PE never starves while
           ScalarE computes exp (keeps the HAM clock-gate warm).
  stage B: flash-style attention in S^T layout ([ts=128, tq<=512] tiles,
           2 heads row-packed per [128,1024] PSUM group, one Exp per
           group on ScalarE, post-exp causal mask multiply on VectorE,
           AV matmuls with a [v | 1] stationary operand (M=65) so row 64
           accumulates the softmax denominator). Diagonal tiles are
           narrowed to tq >= 128*r (causal): less PE + exp work.
           Normalization: ScalarE extracts the denominator row,
           VectorE reciprocal_approx_fast, GPSIMD partition_broadcast,
           VectorE multiply -- all on-chip, no DRAM bounce.
  stage C: output projection
Host-side prep is layout/sharding only (transposes into SBUF-image
layouts, slicing, cast to fp16); all FLOPs run on device.
"""

import sys

for _p in ("/opt/trn_rl_repo",):
    if _p not in sys.path:
        sys.path.insert(0, _p)

import numpy as np

import concourse.bacc as bacc
import concourse.bass as bass
import concourse.tile as tile
from concourse import mybir

F32 = mybir.dt.float32
F16 = mybir.dt.float16

T = 2048
C = 1024
O = 512          # per-core output features (8 heads x 64)
HD = 64
NJB = 4          # tq blocks of 512
NCC = 8          # c chunks of 128
NOC = 4          # o chunks of 128
SCALE = 1.0 / 8.0  # 1/sqrt(64)


def build(mm_mode: str = "f16"):
    sb_dt = F16
    np_dt = np.float16

    nc = bacc.Bacc("TRN2", target_bir_lowering=False, debug=False)

    xt_d = nc.dram_tensor("xt", [NJB, 128, NCC, 512], sb_dt, kind="ExternalInput").ap()
    wqt_d = nc.dram_tensor("wqt", [NOC, 128, NCC, 128], sb_dt, kind="ExternalInput").ap()
    wkt_d = nc.dram_tensor("wkt", [NOC, 128, NCC, 128], sb_dt, kind="ExternalInput").ap()
    wvt_d = nc.dram_tensor("wvt", [NCC, 128, O], sb_dt, kind="ExternalInput").ap()
    wpt_d = nc.dram_tensor("wpt", [NOC, 128, C], sb_dt, kind="ExternalInput").ap()
    bq_d = nc.dram_tensor("bq", [128, NOC], F32, kind="ExternalInput").ap()
    bk_d = nc.dram_tensor("bk", [128, NOC], F32, kind="ExternalInput").ap()
    bvb_d = nc.dram_tensor("bvb", [128, O], F32, kind="ExternalInput").ap()
    mask_d = nc.dram_tensor("masks", [128, 4, 512], sb_dt, kind="ExternalInput").ap()
    out_d = nc.dram_tensor("out", [T, C], F32, kind="ExternalOutput").ap()

    with tile.TileContext(nc) as tc:
        with (
            tc.tile_pool(name="const", bufs=1) as const,
            tc.tile_pool(name="xt_pool", bufs=2) as xt_pool,
            tc.tile_pool(name="qt_pool", bufs=2) as qt_pool,
            tc.tile_pool(name="att_pool", bufs=4) as att_pool,
            tc.tile_pool(name="yt_pool", bufs=2) as yt_pool,
            tc.tile_pool(name="misc", bufs=2) as misc,
            tc.tile_pool(name="bc_pool", bufs=2) as bc_pool,
            tc.tile_pool(name="ost_pool", bufs=3) as ost_pool,
            tc.tile_pool(name="pst", bufs=2, space="PSUM") as pst,
            tc.tile_pool(name="pa", bufs=2, space="PSUM") as pa,
            tc.tile_pool(name="pav", bufs=1, space="PSUM") as pav,
        ):
            # ---- small constants first (they gate early compute) ----
            bq_sb = const.tile([128, NOC], F32, name="bq_sb")
            bk_sb = const.tile([128, NOC], F32, name="bk_sb")
            bvb_sb = const.tile([128, O], F32, name="bvb_sb")
            mask_sb = const.tile([128, 4, 512], sb_dt, name="mask_sb")
            nc.gpsimd.dma_start(out=bq_sb, in_=bq_d)
            nc.gpsimd.dma_start(out=bk_sb, in_=bk_d)
            nc.gpsimd.dma_start(out=bvb_sb, in_=bvb_d)

            # ---- x tiles (per block, two halves) on the sync queue ----
            xt_t = {}

            def load_xt(jb):
                xa = xt_pool.tile([128, 4, 512], sb_dt, tag="xta", name=f"xt{jb}a")
                xb = xt_pool.tile([128, 4, 512], sb_dt, tag="xtb", name=f"xt{jb}b")
                nc.sync.dma_start(out=xa, in_=xt_d[jb, :, 0:4])
                nc.sync.dma_start(out=xb, in_=xt_d[jb, :, 4:8])
                xt_t[jb] = (xa, xb)

            load_xt(0)

            # ---- weights: fine-grained tiles, arrival-ordered to match
            # first use: wq0/wk0, wv (AV of block 0), mask, rest ----
            wq_t = [const.tile([128, NCC, 128], sb_dt, name=f"wq{o}") for o in range(NOC)]
            wk_t = [const.tile([128, NCC, 128], sb_dt, name=f"wk{o}") for o in range(NOC)]
            wv_t = [const.tile([128, O], sb_dt, name=f"wv{c}") for c in range(NCC)]
            wp_t = [const.tile([128, C], sb_dt, name=f"wp{o}") for o in range(NOC)]
            nc.gpsimd.dma_start(out=wq_t[0], in_=wqt_d[0])
            nc.gpsimd.dma_start(out=wk_t[0], in_=wkt_d[0])
            for cc in range(NCC):
                nc.gpsimd.dma_start(out=wv_t[cc], in_=wvt_d[cc])
            nc.gpsimd.dma_start(out=mask_sb, in_=mask_d)
            for oc in range(1, NOC):
                nc.gpsimd.dma_start(out=wq_t[oc], in_=wqt_d[oc])
                nc.gpsimd.dma_start(out=wk_t[oc], in_=wkt_d[oc])
            for oc in range(NOC):
                nc.gpsimd.dma_start(out=wp_t[oc], in_=wpt_d[oc])

            # persistent K^T and V. V carries a ones column per head
            # ([v | 1]) so the AV matmul (M=65) also accumulates the
            # softmax denominator in its row 64.
            kt_t = {}
            v_t = {}
            for jbx in range(NJB):
                for oc in range(NOC):
                    kt_t[oc, jbx] = const.tile(
                        [128, 512], sb_dt, name=f"kt{oc}_{jbx}"
                    )
                v_t[jbx] = const.tile([128, 4, 8, 65], sb_dt, name=f"v_{jbx}")
                nc.vector.memset(v_t[jbx][:, :, :, 64:65], 1.0)

            qt_sb = {}

            def emit_qk(jb, oc):
                # q and k projections for o-chunk oc of block jb;
                # evictions on VectorE (ScalarE is reserved for exp)
                if oc == 0:
                    qt_sb[jb] = qt_pool.tile(
                        [128, NOC, 512], sb_dt, tag="qt", name=f"qt{jb}"
                    )
                xab = xt_t[jb]
                for mat in (0, 1):
                    w = (wq_t if mat == 0 else wk_t)[oc]
                    ps = pa.tile([128, 512], F32, tag="apsum", name=f"qk{jb}{oc}{mat}")
                    for cc in range(NCC):
                        nc.tensor.matmul(
                            ps,
                            lhsT=w[:, cc, :],
                            rhs=xab[cc // 4][:, cc % 4, :],
                            start=(cc == 0),
                            stop=(cc == NCC - 1),
                        )
                    if mat == 0:
                        nc.vector.tensor_scalar(
                            out=qt_sb[jb][:, oc, :], in0=ps,
                            scalar1=SCALE, scalar2=bq_sb[:, oc : oc + 1],
                            op0=mybir.AluOpType.mult, op1=mybir.AluOpType.add,
                        )
                    else:
                        nc.vector.tensor_scalar(
                            out=kt_t[oc, jb], in0=ps,
                            scalar1=1.0, scalar2=bk_sb[:, oc : oc + 1],
                            op0=mybir.AluOpType.mult, op1=mybir.AluOpType.add,
                        )

            def emit_v(jb, tt):
                # v for t-chunk tt: out layout [t-part, o]
                xab = xt_t[jb]
                ps = pa.tile([128, 512], F32, tag="apsum", name=f"v{jb}{tt}")
                for cc in range(NCC):
                    nc.tensor.matmul(
                        ps,
                        lhsT=xab[cc // 4][:, cc % 4, 128 * tt : 128 * (tt + 1)],
                        rhs=wv_t[cc],
                        start=(cc == 0),
                        stop=(cc == NCC - 1),
                    )
                nc.vector.scalar_tensor_tensor(
                    v_t[jb][:, tt, :, 0:64],
                    ps.rearrange("p (h d) -> p h d", h=8),
                    0.0,
                    bvb_sb.rearrange("p (h d) -> p h d", h=8),
                    op0=mybir.AluOpType.add,
                    op1=mybir.AluOpType.add,
                )

            # PE filler: stage-A chains for the next block, drip-fed into
            # the attention loop so the PE queue never drains while ACT
            # works through the exps.
            filler = []

            def pop_filler():
                if filler:
                    filler.pop(0)()

            def attn_pair(jb, p, yt_c, fill_every):
                avp = pav.tile([65, 1024], F32, tag="av", name=f"av{jb}{p}")
                n_ts = 4 * jb + 4
                for tsb in range(n_ts):
                    first = tsb == 0
                    last = tsb == n_ts - 1
                    diag = tsb >= 4 * jb
                    r = tsb - 4 * jb
                    lo = 128 * r if diag else 0  # causal narrowing
                    st = pst.tile([128, 1024], F32, tag="st")
                    for r2 in range(2):
                        nc.tensor.matmul(
                            st[:, 512 * r2 + lo : 512 * (r2 + 1)],
                            lhsT=kt_t[p, tsb // 4][
                                64 * r2 : 64 * (r2 + 1),
                                128 * (tsb % 4) : 128 * (tsb % 4 + 1),
                            ],
                            rhs=qt_sb[jb][64 * r2 : 64 * (r2 + 1), p, lo:512],
                            tile_position=(64 * r2, 0),
                            start=True,
                            stop=True,
                        )
                    att = att_pool.tile([128, 1024], sb_dt, tag="att")
                    st3 = st.rearrange("p (h q) -> p h q", h=2)
                    att3 = att.rearrange("p (h q) -> p h q", h=2)
                    nc.scalar.activation(
                        att3[:, :, lo:512], st3[:, :, lo:512],
                        mybir.ActivationFunctionType.Exp,
                    )
                    if diag:
                        for r2 in range(2):
                            sl5 = slice(512 * r2 + lo, 512 * (r2 + 1))
                            nc.vector.tensor_mul(
                                att[:, sl5], att[:, sl5], mask_sb[:, r, lo:512]
                            )
                    for r2 in range(2):
                        h = 2 * p + r2
                        nc.tensor.matmul(
                            avp[0:65, 512 * r2 + lo : 512 * (r2 + 1)],
                            lhsT=v_t[tsb // 4][:, tsb % 4, h, :],
                            rhs=att[:, 512 * r2 + lo : 512 * (r2 + 1)],
                            start=first,
                            stop=last,
                        )
                    if tsb % fill_every == fill_every - 1:
                        pop_filler()
                # ---- normalization (all on-chip) ----
                den = misc.tile([1, 1024], F32, tag="den")
                rden = misc.tile([1, 1024], F32, tag="rden")
                nc.vector.tensor_copy(den, avp[64:65, :])
                nc.vector.reciprocal_approx_fast(rden, den)
                bc = bc_pool.tile([64, 1024], F32, tag="bc")
                nc.gpsimd.partition_broadcast(bc, rden, channels=64)
                nc.vector.tensor_mul(yt_c[p][0:64, :], avp[0:64, 0:512], bc[:, 0:512])
                nc.vector.tensor_mul(
                    yt_c[p][64:128, :], avp[0:64, 512:1024], bc[:, 512:1024]
                )

            def stage_c_chain(jb, yt_c, cb, tt):
                op = pa.tile([128, 512], F32, tag="apsum", name=f"c{cb}{tt}")
                for oc in range(NOC):
                    nc.tensor.matmul(
                        op,
                        lhsT=yt_c[oc][:, 128 * tt : 128 * (tt + 1)],
                        rhs=wp_t[oc][:, 512 * cb : 512 * (cb + 1)],
                        start=(oc == 0),
                        stop=(oc == NOC - 1),
                    )
                ost = ost_pool.tile([128, 512], F32, tag="ost")
                nc.vector.tensor_copy(ost, op)
                nc.sync.dma_start(
                    out=out_d[
                        512 * jb + 128 * tt : 512 * jb + 128 * (tt + 1),
                        512 * cb : 512 * (cb + 1),
                    ],
                    in_=ost,
                )

            # ---- prologue: stage A for block 0 (v early: AV needs it) ----
            emit_qk(0, 0)
            for tt in range(4):
                emit_v(0, tt)
            for oc in range(1, NOC):
                emit_qk(0, oc)

            prev = None  # (jb-1, yt_c of jb-1): stage C deferred as filler
            for jb in range(NJB):
                a_chains = []
                if jb + 1 < NJB:
                    load_xt(jb + 1)
                    a_chains = [
                        lambda jbn=jb + 1, o=oc: emit_qk(jbn, o) for oc in range(NOC)
                    ] + [lambda jbn=jb + 1, t=tt: emit_v(jbn, t) for tt in range(4)]
                c_chains = []
                if prev is not None:
                    pj, pyt = prev
                    c_chains = [
                        lambda j=pj, y=pyt, c=cb, t=tt: stage_c_chain(j, y, c, t)
                        for cb in range(2)
                        for tt in range(4)
                    ]
                # interleave next-block projections with prev-block output proj
                while a_chains or c_chains:
                    filler.extend(a_chains[:2])
                    del a_chains[:2]
                    filler.extend(c_chains[:1])
                    del c_chains[:1]
                yt_c = [
                    yt_pool.tile([128, 512], sb_dt, tag=f"yt{_o}", name=f"yt{_o}")
                    for _o in range(NOC)
                ]
                n_iters = 4 * (4 * jb + 4)
                fill_every = max(1, n_iters // max(1, len(filler)))
                for p in range(NOC):
                    attn_pair(jb, p, yt_c, fill_every)
                while filler:
                    pop_filler()
                prev = (jb, yt_c)
            # final block's output projection (tail)
            pj, pyt = prev
            for cb in range(2):
                for tt in range(4):
                    stage_c_chain(pj, pyt, cb, tt)

    nc.finalize()
    return nc, {"np_dt": np_dt}


def make_masks(np_dt):
    """masks[r][p, n] = 1 if n >= 128*r + p else 0."""
    n = np.arange(512)[None, :]
    p = np.arange(128)[:, None]
    out = np.zeros((4, 128, 512), np.float32)
    for r in range(4):
        out[r] = (n >= 128 * r + p).astype(np.float32)
    return out.astype(np_dt)


def shard_inputs(inputs, np_dt):
    """Full inputs -> list of 8 per-core input dicts."""
    q = np.asarray(inputs["query"], np.float32)
    Wq = np.asarray(inputs["Wq"], np.float32)
    Wk = np.asarray(inputs["Wk"], np.float32)
    Wv = np.asarray(inputs["Wv"], np.float32)
    Wp = np.asarray(inputs["Wp"], np.float32)
    bq = np.asarray(inputs["bq"], np.float32)
    bk = np.asarray(inputs["bk"], np.float32)
    bv = np.asarray(inputs["bv"], np.float32)
    masks = np.ascontiguousarray(make_masks(np_dt).transpose(1, 0, 2))
    in_maps = []
    for core in range(8):
        b, g = core // 2, core % 2
        sl = slice(O * g, O * (g + 1))
        in_maps.append({
            "xt": np.ascontiguousarray(
                q[b].T.reshape(8, 128, NJB, 512).transpose(2, 1, 0, 3)
            ).astype(np_dt),
            # [oc, p, cc, j]: lhsT chunk for (oc, cc) = W.T[128cc:+128, 128oc:+128]
            "wqt": np.ascontiguousarray(
                Wq[sl, :].T.reshape(NCC, 128, NOC, 128).transpose(2, 1, 0, 3)
            ).astype(np_dt),
            "wkt": np.ascontiguousarray(
                Wk[sl, :].T.reshape(NCC, 128, NOC, 128).transpose(2, 1, 0, 3)
            ).astype(np_dt),
            # [cc, p, o]
            "wvt": np.ascontiguousarray(
                Wv[sl, :].T.reshape(NCC, 128, O)
            ).astype(np_dt),
            # [oc, p, c]
            "wpt": np.ascontiguousarray(
                Wp[:, sl].T.reshape(NOC, 128, C)
            ).astype(np_dt),
            "bq": np.ascontiguousarray(bq[sl].reshape(NOC, 128).T) * np.float32(SCALE),
            "bk": np.ascontiguousarray(bk[sl].reshape(NOC, 128).T),
            "bvb": np.broadcast_to(bv[sl], (128, O)).copy(),
            "masks": masks,
        })
    return in_maps


def unshard(results, bp):
    out = np.empty((4, T, C), np.float32)
    for b in range(4):
        out[b] = results[2 * b]["out"] + results[2 * b + 1]["out"] + np.asarray(
            bp, np.float32
        )
    return out


_CACHE = {}


def _get_nc(mode="f16"):
    if mode not in _CACHE:
        _CACHE[mode] = build(mode)
    return _CACHE[mode]


def kernel(**inputs):
    """Full unsharded inputs -> full [4, 2048, 1024] fp32 output."""
    from concourse import bass_utils

    nc, meta = _get_nc("f16")
    in_maps = shard_inputs(inputs, meta["np_dt"])
    res = bass_utils.run_bass_kernel_spmd(nc, in_maps, list(range(8)))
    return unshard(res.results, inputs["bp"])


# revision 10
# speedup vs baseline: 1.2897x; 1.0514x over previous
"""Self-contained Trainium2 Bass kernel: causal self-attention, 8-core SPMD.

nn_CausalSelfAttention: B=4, T=2048, C=1024, n_head=16 (fp32 reference).

Sharding (hardcoded): core c -> batch b = c//2, head-group g = c%2
(8 of 16 heads = 512 features). Data parallel over B, tensor parallel
over heads. Each core computes a partial output [T, C] = y_g @ Wp_g^T;
the host sums the two partials per batch and adds bp (the tensor-parallel
all-reduce done at unshard time).

Device kernel (per core), v2:
  stage A: QKV projections (fp16 matmuls, fp32 PSUM accumulation).
           Block jb+1's projections are emitted as PE filler interleaved
           into block jb's attention loop so the PE never starves while
           ScalarE computes exp (keeps the HAM clock-gate warm).
  stage B: flash-style attention in S^T layout ([ts=128, tq<=512] tiles,
           2 heads row-packed per [128,1024] PSUM group, one Exp per
           group on ScalarE, post-exp causal mask multiply on VectorE,
           AV matmuls with a [v | 1] stationary operand (M=65) so row 64
           accumulates the softmax denominator). Diagonal tiles are
           narrowed to tq >= 128*r (causal): less PE + exp work.
           Normalization: ScalarE extracts the denominator row,
           VectorE reciprocal_approx_fast, GPSIMD partition_broadcast,
           VectorE multiply -- all on-chip, no DRAM bounce.
  stage C: output projection
Host-side prep is layout/sharding only (transposes into SBUF-image
layouts, slicing, cast to fp16); all FLOPs run on device.
"""

import sys

for _p in ("/opt/trn_rl_repo",):
    if _p not in sys.path:
        sys.path.insert(0, _p)

import numpy as np

import concourse.bacc as bacc
import concourse.bass as bass
import concourse.tile as tile
from concourse import mybir

F32 = mybir.dt.float32
F16 = mybir.dt.float16

T = 2048
C = 1024
O = 512          # per-core output features (8 heads x 64)
HD = 64
NJB = 4          # tq blocks of 512
NCC = 8          # c chunks of 128
NOC = 4          # o chunks of 128
SCALE = 1.0 / 8.0  # 1/sqrt(64)


def build(mm_mode: str = "f16"):
    sb_dt = F16
    np_dt = np.float16

    nc = bacc.Bacc("TRN2", target_bir_lowering=False, debug=False)

    xt_d = nc.dram_tensor("xt", [NJB, 128, NCC, 512], sb_dt, kind="ExternalInput").ap()
    wq0_d = nc.dram_tensor("wq0", [128, NCC, 128], sb_dt, kind="ExternalInput").ap()
    wk0_d = nc.dram_tensor("wk0", [128, NCC, 128], sb_dt, kind="ExternalInput").ap()
    wqr_d = nc.dram_tensor("wqr", [128, 3, NCC, 128], sb_dt, kind="ExternalInput").ap()
    wkr_d = nc.dram_tensor("wkr", [128, 3, NCC, 128], sb_dt, kind="ExternalInput").ap()
    wvt_d = nc.dram_tensor("wvt", [128, NCC, O], sb_dt, kind="ExternalInput").ap()
    wpt_d = nc.dram_tensor("wpt", [128, NOC, C], sb_dt, kind="ExternalInput").ap()
    # bias: cols [0:4]=bq*SCALE, [4:8]=bk
    bias_d = nc.dram_tensor("bias", [128, 8], F32, kind="ExternalInput").ap()
    bvb_d = nc.dram_tensor("bvb", [128, O], F32, kind="ExternalInput").ap()
    mask_d = nc.dram_tensor("masks", [128, 4, 512], sb_dt, kind="ExternalInput").ap()
    out_d = nc.dram_tensor("out", [T, C], F32, kind="ExternalOutput").ap()

    with tile.TileContext(nc) as tc:
        with (
            tc.tile_pool(name="const", bufs=1) as const,
            tc.tile_pool(name="xt_pool", bufs=2) as xt_pool,
            tc.tile_pool(name="qt_pool", bufs=2) as qt_pool,
            tc.tile_pool(name="att_pool", bufs=5) as att_pool,
            tc.tile_pool(name="yt_pool", bufs=2) as yt_pool,
            tc.tile_pool(name="misc", bufs=2) as misc,
            tc.tile_pool(name="bc_pool", bufs=2) as bc_pool,
            tc.tile_pool(name="ost_pool", bufs=3) as ost_pool,
            tc.tile_pool(name="pst", bufs=2, space="PSUM") as pst,
            tc.tile_pool(name="pa", bufs=2, space="PSUM") as pa,
            tc.tile_pool(name="pav", bufs=1, space="PSUM") as pav,
        ):
            # ---- preload: few large DMAs, spread across idle queues ----
            bias_sb = const.tile([128, 8], F32, name="bias_sb")
            bvb_sb = const.tile([128, O], F32, name="bvb_sb")
            mask_sb = const.tile([128, 4, 512], sb_dt, name="mask_sb")

            xt_t = {}

            def load_xt(jb):
                xa = xt_pool.tile([128, 4, 512], sb_dt, tag="xta", name=f"xt{jb}a")
                xb = xt_pool.tile([128, 4, 512], sb_dt, tag="xtb", name=f"xt{jb}b")
                nc.sync.dma_start(out=xa, in_=xt_d[jb, :, 0:4])
                nc.sync.dma_start(out=xb, in_=xt_d[jb, :, 4:8])
                xt_t[jb] = (xa, xb)

            wq0_sb = const.tile([128, NCC, 128], sb_dt, name="wq0_sb")
            wk0_sb = const.tile([128, NCC, 128], sb_dt, name="wk0_sb")
            wqr_sb = const.tile([128, 3, NCC, 128], sb_dt, name="wqr_sb")
            wkr_sb = const.tile([128, 3, NCC, 128], sb_dt, name="wkr_sb")
            wv_sb = const.tile([128, NCC, O], sb_dt, name="wv_sb")
            wp_sb = const.tile([128, NOC, C], sb_dt, name="wp_sb")
            # pool queue: first-needed weights
            nc.gpsimd.dma_start(out=bias_sb, in_=bias_d)
            nc.gpsimd.dma_start(out=bvb_sb, in_=bvb_d)
            nc.gpsimd.dma_start(out=wq0_sb, in_=wq0_d)
            nc.gpsimd.dma_start(out=wk0_sb, in_=wk0_d)
            nc.gpsimd.dma_start(out=wqr_sb, in_=wqr_d)
            nc.gpsimd.dma_start(out=wkr_sb, in_=wkr_d)
            # sync queue: x for block 0, then mask
            load_xt(0)
            nc.sync.dma_start(out=mask_sb, in_=mask_d)
            # act queue (idle at start): v/p weights (needed a few us in)
            nc.scalar.dma_start(out=wv_sb, in_=wvt_d)
            nc.scalar.dma_start(out=wp_sb, in_=wpt_d)

            def wq_t(oc):
                return wq0_sb if oc == 0 else wqr_sb[:, oc - 1]

            def wk_t(oc):
                return wk0_sb if oc == 0 else wkr_sb[:, oc - 1]

            # persistent K^T and V. V carries a ones column per head
            # ([v | 1]) so the AV matmul (M=65) also accumulates the
            # softmax denominator in its row 64.
            kt_t = {}
            v_t = {}
            for jbx in range(NJB):
                for oc in range(NOC):
                    kt_t[oc, jbx] = const.tile(
                        [128, 512], sb_dt, name=f"kt{oc}_{jbx}"
                    )
                v_t[jbx] = const.tile([128, 4, 8, 65], sb_dt, name=f"v_{jbx}")
                nc.vector.memset(v_t[jbx][:, :, :, 64:65], 1.0)

            qt_sb = {}

            def emit_qk(jb, oc):
                # q and k projections for o-chunk oc of block jb;
                # evictions on VectorE (ScalarE is reserved for exp)
                if oc == 0:
                    qt_sb[jb] = qt_pool.tile(
                        [128, NOC, 512], sb_dt, tag="qt", name=f"qt{jb}"
                    )
                xab = xt_t[jb]
                for mat in (0, 1):
                    w = (wq_t if mat == 0 else wk_t)(oc)
                    ps = pa.tile([128, 512], F32, tag="apsum", name=f"qk{jb}{oc}{mat}")
                    for cc in range(NCC):
                        nc.tensor.matmul(
                            ps,
                            lhsT=w[:, cc, :],
                            rhs=xab[cc // 4][:, cc % 4, :],
                            start=(cc == 0),
                            stop=(cc == NCC - 1),
                        )
                    if mat == 0:
                        nc.vector.tensor_scalar(
                            out=qt_sb[jb][:, oc, :], in0=ps,
                            scalar1=SCALE, scalar2=bias_sb[:, oc : oc + 1],
                            op0=mybir.AluOpType.mult, op1=mybir.AluOpType.add,
                        )
                    else:
                        nc.vector.tensor_scalar(
                            out=kt_t[oc, jb], in0=ps,
                            scalar1=1.0, scalar2=bias_sb[:, 4 + oc : 5 + oc],
                            op0=mybir.AluOpType.mult, op1=mybir.AluOpType.add,
                        )

            def emit_v(jb, tt):
                # v for t-chunk tt: out layout [t-part, o]
                xab = xt_t[jb]
                ps = pa.tile([128, 512], F32, tag="apsum", name=f"v{jb}{tt}")
                for cc in range(NCC):
                    nc.tensor.matmul(
                        ps,
                        lhsT=xab[cc // 4][:, cc % 4, 128 * tt : 128 * (tt + 1)],
                        rhs=wv_sb[:, cc, :],
                        start=(cc == 0),
                        stop=(cc == NCC - 1),
                    )
                nc.vector.scalar_tensor_tensor(
                    v_t[jb][:, tt, :, 0:64],
                    ps.rearrange("p (h d) -> p h d", h=8),
                    0.0,
                    bvb_sb.rearrange("p (h d) -> p h d", h=8),
                    op0=mybir.AluOpType.add,
                    op1=mybir.AluOpType.add,
                )

            # PE filler: stage-A chains for the next block, drip-fed into
            # the attention loop so the PE queue never drains while ACT
            # works through the exps.
            filler = []

            def pop_filler():
                if filler:
                    filler.pop(0)()

            def attn_pair(jb, p, yt_c, fill_every):
                avp = pav.tile([65, 1024], F32, tag="av", name=f"av{jb}{p}")
                n_ts = 4 * jb + 4
                for tsb in range(n_ts):
                    first = tsb == 0
                    last = tsb == n_ts - 1
                    diag = tsb >= 4 * jb
                    r = tsb - 4 * jb
                    lo = 128 * r if diag else 0  # causal narrowing
                    st = pst.tile([128, 1024], F32, tag="st")
                    for r2 in range(2):
                        nc.tensor.matmul(
                            st[:, 512 * r2 + lo : 512 * (r2 + 1)],
                            lhsT=kt_t[p, tsb // 4][
                                64 * r2 : 64 * (r2 + 1),
                                128 * (tsb % 4) : 128 * (tsb % 4 + 1),
                            ],
                            rhs=qt_sb[jb][64 * r2 : 64 * (r2 + 1), p, lo:512],
                            tile_position=(64 * r2, 0),
                            start=True,
                            stop=True,
                        )
                    att = att_pool.tile([128, 1024], sb_dt, tag="att")
                    st3 = st.rearrange("p (h q) -> p h q", h=2)
                    att3 = att.rearrange("p (h q) -> p h q", h=2)
                    nc.scalar.activation(
                        att3[:, :, lo:512], st3[:, :, lo:512],
                        mybir.ActivationFunctionType.Exp,
                    )
                    if diag:
                        for r2 in range(2):
                            sl5 = slice(512 * r2 + lo, 512 * (r2 + 1))
                            nc.vector.tensor_mul(
                                att[:, sl5], att[:, sl5], mask_sb[:, r, lo:512]
                            )
                    for r2 in range(2):
                        h = 2 * p + r2
                        nc.tensor.matmul(
                            avp[0:65, 512 * r2 + lo : 512 * (r2 + 1)],
                            lhsT=v_t[tsb // 4][:, tsb % 4, h, :],
                            rhs=att[:, 512 * r2 + lo : 512 * (r2 + 1)],
                            start=first,
                            stop=last,
                        )
                    if tsb % fill_every == fill_every - 1:
                        pop_filler()
                # ---- normalization (all on-chip) ----
                # one fast PSUM->SBUF eviction frees the AV accumulator for
                # the next pair; the rest of the chain runs off SBUF.
                yra = misc.tile([65, 1024], F32, tag="yra")
                nc.vector.tensor_copy(yra, avp[0:65, :])
                den = misc.tile([1, 1024], F32, tag="den")
                rden = misc.tile([1, 1024], F32, tag="rden")
                nc.vector.tensor_copy(den, yra[64:65, :])
                nc.vector.reciprocal_approx_fast(rden, den)
                bc = bc_pool.tile([64, 1024], F32, tag="bc")
                nc.gpsimd.partition_broadcast(bc, rden, channels=64)
                nc.vector.tensor_mul(yt_c[p][0:64, :], yra[0:64, 0:512], bc[:, 0:512])
                nc.vector.tensor_mul(
                    yt_c[p][64:128, :], yra[0:64, 512:1024], bc[:, 512:1024]
                )

            def stage_c_chain(jb, yt_c, cb, tt):
                op = pa.tile([128, 512], F32, tag="apsum", name=f"c{cb}{tt}")
                for oc in range(NOC):
                    nc.tensor.matmul(
                        op,
                        lhsT=yt_c[oc][:, 128 * tt : 128 * (tt + 1)],
                        rhs=wp_sb[:, oc, 512 * cb : 512 * (cb + 1)],
                        start=(oc == 0),
                        stop=(oc == NOC - 1),
                    )
                ost = ost_pool.tile([128, 512], F32, tag="ost")
                nc.vector.tensor_copy(ost, op)
                nc.sync.dma_start(
                    out=out_d[
                        512 * jb + 128 * tt : 512 * jb + 128 * (tt + 1),
                        512 * cb : 512 * (cb + 1),
                    ],
                    in_=ost,
                )

            # ---- prologue: stage A for block 0 (v early: AV needs it) ----
            emit_qk(0, 0)
            for tt in range(4):
                emit_v(0, tt)
            for oc in range(1, NOC):
                emit_qk(0, oc)

            prev = None  # (jb-1, yt_c of jb-1): stage C deferred as filler
            for jb in range(NJB):
                a_chains = []
                if jb + 1 < NJB:
                    load_xt(jb + 1)
                    a_chains = [
                        lambda jbn=jb + 1, o=oc: emit_qk(jbn, o) for oc in range(NOC)
                    ] + [lambda jbn=jb + 1, t=tt: emit_v(jbn, t) for tt in range(4)]
                c_chains = []
                if prev is not None:
                    pj, pyt = prev
                    c_chains = [
                        lambda j=pj, y=pyt, c=cb, t=tt: stage_c_chain(j, y, c, t)
                        for cb in range(2)
                        for tt in range(4)
                    ]
                # interleave next-block projections with prev-block output proj
                while a_chains or c_chains:
                    filler.extend(a_chains[:2])
                    del a_chains[:2]
                    filler.extend(c_chains[:1])
                    del c_chains[:1]
                yt_c = [
                    yt_pool.tile([128, 512], sb_dt, tag=f"yt{_o}", name=f"yt{_o}")
                    for _o in range(NOC)
                ]
                n_iters = 4 * (4 * jb + 4)
                fill_every = max(1, n_iters // max(1, len(filler)))
                for p in range(NOC):
                    attn_pair(jb, p, yt_c, fill_every)
                while filler:
                    pop_filler()
                prev = (jb, yt_c)
            # final block's output projection (tail)
            pj, pyt = prev
            for cb in range(2):
                for tt in range(4):
                    stage_c_chain(pj, pyt, cb, tt)

    nc.finalize()
    return nc, {"np_dt": np_dt}


def make_masks(np_dt):
    """masks[r][p, n] = 1 if n >= 128*r + p else 0."""
    n = np.arange(512)[None, :]
    p = np.arange(128)[:, None]
    out = np.zeros((4, 128, 512), np.float32)
    for r in range(4):
        out[r] = (n >= 128 * r + p).astype(np.float32)
    return out.astype(np_dt)


def shard_inputs(inputs, np_dt):
    """Full inputs -> list of 8 per-core input dicts."""
    q = np.asarray(inputs["query"], np.float32)
    Wq = np.asarray(inputs["Wq"], np.float32)
    Wk = np.asarray(inputs["Wk"], np.float32)
    Wv = np.asarray(inputs["Wv"], np.float32)
    Wp = np.asarray(inputs["Wp"], np.float32)
    bq = np.asarray(inputs["bq"], np.float32)
    bk = np.asarray(inputs["bk"], np.float32)
    bv = np.asarray(inputs["bv"], np.float32)
    masks = np.ascontiguousarray(make_masks(np_dt).transpose(1, 0, 2))
    in_maps = []
    for core in range(8):
        b, g = core // 2, core % 2
        sl = slice(O * g, O * (g + 1))
        # [p, oc, cc, j]: lhsT chunk for (oc, cc) = W.T[128cc:+128, 128oc:+128]
        wq4 = Wq[sl, :].T.reshape(NCC, 128, NOC, 128).transpose(1, 2, 0, 3)
        wk4 = Wk[sl, :].T.reshape(NCC, 128, NOC, 128).transpose(1, 2, 0, 3)
        bias = np.empty((128, 8), np.float32)
        bias[:, 0:4] = bq[sl].reshape(NOC, 128).T * np.float32(SCALE)
        bias[:, 4:8] = bk[sl].reshape(NOC, 128).T
        in_maps.append({
            "xt": np.ascontiguousarray(
                q[b].T.reshape(8, 128, NJB, 512).transpose(2, 1, 0, 3)
            ).astype(np_dt),
            "wq0": np.ascontiguousarray(wq4[:, 0]).astype(np_dt),
            "wk0": np.ascontiguousarray(wk4[:, 0]).astype(np_dt),
            "wqr": np.ascontiguousarray(wq4[:, 1:4]).astype(np_dt),
            "wkr": np.ascontiguousarray(wk4[:, 1:4]).astype(np_dt),
            # [p, cc, o]
            "wvt": np.ascontiguousarray(
                Wv[sl, :].T.reshape(NCC, 128, O).transpose(1, 0, 2)
            ).astype(np_dt),
            # [p, oc, c]
            "wpt": np.ascontiguousarray(
                Wp[:, sl].T.reshape(NOC, 128, C).transpose(1, 0, 2)
            ).astype(np_dt),
            "bias": bias,
            "bvb": np.broadcast_to(bv[sl], (128, O)).copy(),
            "masks": masks,
        })
    return in_maps


def unshard(results, bp):
    out = np.empty((4, T, C), np.float32)
    for b in range(4):
        out[b] = results[2 * b]["out"] + results[2 * b + 1]["out"] + np.asarray(
            bp, np.float32
        )
    return out


_CACHE = {}


def _get_nc(mode="f16"):
    if mode not in _CACHE:
        _CACHE[mode] = build(mode)
    return _CACHE[mode]


def kernel(**inputs):
    """Full unsharded inputs -> full [4, 2048, 1024] fp32 output."""
    from concourse import bass_utils

    nc, meta = _get_nc("f16")
    in_maps = shard_inputs(inputs, meta["np_dt"])
    res = bass_utils.run_bass_kernel_spmd(nc, in_maps, list(range(8)))
    return unshard(res.results, inputs["bp"])
